# revision 1
# baseline (speedup 1.0000x reference)
"""LCA layer kernel for Trainium2, data-parallel over tokens on 8 NeuronCores.

Reference (per token row x of d_model=1024, W [1024, 4096]):
    b = x @ W;  G = W^T W with zero diag;  u_0 = 0
    10x: a = relu(u - 0.1); u = 0.9 u + 0.1 (b - a @ G - u)
    out = relu(u - 0.1) @ W^T

Device algorithm (per core, 1024 tokens, 4 blocks of T=256):
  * factor a@G = (a @ W^T) @ W - g * a  (g = diag(W^T W)), avoiding G.
  * the 18 inner matmul passes (a@W^T and h@W01) run in fp8-e4m3 with
    perf_mode=DoubleRowSwInterleave (measured ~2.4x over bf16 on HW).
  * instead of a, the fp8 activation tensor holds ga8 = Sga*(g*a), produced
    directly by one ACT relu per chunk (g*relu(x) == relu(g*x), with
    per-partition AP scale/bias); 1/g is folded into the W^T fp8 weights
    host-side. This removes the per-chunk g scalar from the DVE update,
    so all DVE/Pool elementwise ops run on [P,512] chunk-pairs.
  * update u' = 0.9u + (B' + g*a - Y') via STT1 (DVE, psum), TT (Pool),
    STT3 (DVE, lagged 2 pairs to hide the cross-engine round trip), and
    one ACT relu -> next ga8. u, B' bf16, SHSY-scaled.
  * ht matmuls for step k+1 are fused into step k's Y-phase loop.
  * B' = x@(0.1W) and out = a@W^T stay bf16 (their errors enter the result
    coherently); their bf16 weights are streamed from HBM per block.
  * PSUM banks are shared by two accumulators; only the first matmul
    touching a bank uses start=True (ZERO_REGION is the whole 2KB bank),
    the second half's first write relies on pending-zero overwrite.
  * emulated end-to-end rel_l2 vs fp32 reference: ~8.4e-3.
"""

import numpy as np
import ml_dtypes

P = 128
T = 256
NBLK = 4
NSTEPS = 9        # iterated steps (step 1 is the free u=B' init)
DM = 1024
DL = 4096
NDM = DM // P     # 8
NDL = DL // P     # 32
NPAIR_H = NDL // 2   # 16 fp8 contraction pairs for ht
NPAIR_Y = NDM // 2   # 4 pairs for Y'
NCORES = 8
TOK_CORE = NBLK * T

SGA = 128.0       # ga8 = SGA * (g * a)
SW2 = 2.0         # wti = SW2 * W^T / g
SH = 8.0          # ht8 = SH * h
SY = 512.0        # w01i = -SY * 0.1 * W
SHSY = SH * SY    # 4096; u and B' are stored SHSY-scaled
C_HT = SH / (SGA * SW2)   # psum_h -> ht8
C_T = SHSY / SGA          # ga8 -> SHSY*(g*a) in STT1

BF16 = ml_dtypes.bfloat16
F8 = ml_dtypes.float8_e4m3

_CACHE = {}
TRACE = False
LAST_RESULT = None


def _build_nc(nsteps=NSTEPS, nrep=1):
    import concourse.bacc as bacc
    import concourse.tile as tile
    import concourse.mybir as mybir

    dt = mybir.dt
    Alu = mybir.AluOpType
    Act = mybir.ActivationFunctionType
    SWI = mybir.MatmulPerfMode.DoubleRowSwInterleave

    nc = bacc.Bacc("TRN2", target_bir_lowering=False, debug=False,
                   num_devices=NCORES)

    xt_d = nc.dram_tensor("xt", [NBLK, P, NDM, T], dt.bfloat16,
                          kind="ExternalInput").ap()
    wti_d = nc.dram_tensor("wti", [P, NPAIR_H, NDM, 2 * P], dt.float8e4,
                           kind="ExternalInput").ap()
    w01i_d = nc.dram_tensor("w01i", [P, NPAIR_Y, NDL, 2 * P], dt.float8e4,
                            kind="ExternalInput").ap()
    w01b_d = nc.dram_tensor("w01b", [P, 4, NDM, 1024], dt.bfloat16,
                            kind="ExternalInput").ap()
    wtb_d = nc.dram_tensor("wtb", [P, NDL, DM], dt.bfloat16,
                           kind="ExternalInput").ap()
    gsb_d = nc.dram_tensor("gsb", [P, NDL], dt.float32,
                           kind="ExternalInput").ap()
    gss_d = nc.dram_tensor("gss", [P, NDL], dt.float32,
                           kind="ExternalInput").ap()
    gbi_d = nc.dram_tensor("gbi", [P, NDL], dt.float32,
                           kind="ExternalInput").ap()
    out_d = nc.dram_tensor("out", [TOK_CORE, DM], dt.float32,
                           kind="ExternalOutput").ap()

    with tile.TileContext(nc) as tc:
        with (
            tc.tile_pool(name="wpool", bufs=1) as wpool,
            tc.tile_pool(name="w01s", bufs=2) as w01s,
            tc.tile_pool(name="wtbs", bufs=3) as wtbs,
            tc.tile_pool(name="state", bufs=1) as state,
            tc.tile_pool(name="htp", bufs=2) as htp,
            tc.tile_pool(name="xio", bufs=2) as xio,
            tc.tile_pool(name="oio", bufs=2) as oio,
            tc.tile_pool(name="ttmp", bufs=4) as ttmp,
            tc.tile_pool(name="stmp", bufs=4) as stmp,
            tc.tile_pool(name="apsum", bufs=4, space="PSUM") as apsum,
            tc.tile_pool(name="ypsum", bufs=4, space="PSUM") as ypsum,
        ):
            # ---- resident fp8 weights + per-feature g vectors ----
            wti = wpool.tile([P, NPAIR_H, NDM, 2 * P], dt.float8e4, tag="wti")
            w01i = wpool.tile([P, NPAIR_Y, NDL, 2 * P], dt.float8e4, tag="w01i")
            gsb = wpool.tile([P, NDL], dt.float32, tag="gsb")
            gss = wpool.tile([P, NDL], dt.float32, tag="gss")
            gbi = wpool.tile([P, NDL], dt.float32, tag="gbi")
            nl_ab = wpool.tile([P, 1], dt.float32, tag="nl_ab")
            nc.gpsimd.memset(nl_ab[:], -0.1)
            # block-0 inputs first so B' isn't stuck behind the 8MB of fp8
            # weights in the SP DMA queue
            xt0 = xio.tile([P, NDM, T], dt.bfloat16, tag="xt")
            wg0 = w01s.tile([P, NDM, 1024], dt.bfloat16, tag="wg")
            # interleave fine-grained first-block DMAs so the B' matmuls can
            # start as soon as the first dm-chunk slices land
            nc.sync.dma_start(xt0[:, 0:2], xt_d[0][:, 0:2])
            nc.sync.dma_start(wg0[:, 0:1], w01b_d[:, 0, 0:1])
            nc.sync.dma_start(wg0[:, 1:2], w01b_d[:, 0, 1:2])
            nc.sync.dma_start(xt0[:, 2:4], xt_d[0][:, 2:4])
            nc.sync.dma_start(wg0[:, 2:4], w01b_d[:, 0, 2:4])
            nc.sync.dma_start(xt0[:, 4:8], xt_d[0][:, 4:8])
            nc.sync.dma_start(wg0[:, 4:6], w01b_d[:, 0, 4:6])
            nc.sync.dma_start(wg0[:, 6:8], w01b_d[:, 0, 6:8])
            nc.sync.dma_start(gsb[:], gsb_d[:])
            nc.sync.dma_start(gss[:], gss_d[:])
            nc.sync.dma_start(gbi[:], gbi_d[:])
            for kg in range(0, NPAIR_H, 4):
                nc.sync.dma_start(wti[:, kg:kg + 4], wti_d[:, kg:kg + 4])
            for kg in range(NPAIR_Y):
                nc.sync.dma_start(w01i[:, kg], w01i_d[:, kg])

            for rep in range(nrep):
              for blk in range(NBLK):
                if blk == 0 and rep == 0:
                    xt = xt0
                else:
                    xt = xio.tile([P, NDM, T], dt.bfloat16, tag="xt")
                    nc.sync.dma_start(xt[:], xt_d[blk])

                u = state.tile([P, NDL, T], dt.bfloat16, tag="u")
                bps = state.tile([P, NDL, T], dt.bfloat16, tag="bps")
                ga8 = state.tile([P, NDL, T], dt.float8e4, tag="ga8")

                # ---- B' = x @ W01 (transposed out), bf16 weights streamed;
                # half-grps of 4 chunks on 2 psum banks pipeline the drains
                for grp in range(4):
                    if blk == 0 and grp == 0 and rep == 0:
                        wg = wg0
                    else:
                        wg = w01s.tile([P, NDM, 1024], dt.bfloat16, tag="wg")
                        for h in range(0, NDM, 2):
                            nc.sync.dma_start(wg[:, h:h + 2],
                                              w01b_d[:, grp, h:h + 2])
                    for half in range(2):
                        pbs = []
                        for _ in range(2):
                            pb = apsum.tile([P, 2 * T], dt.float32, tag="mm")
                            pbs.append(pb)
                        def pbv(jj):
                            return pbs[jj // 2][:, (jj % 2) * T:(jj % 2 + 1) * T]
                        j0 = half * 4
                        for dmc in range(NDM):
                            for jj in range(4):
                                nc.tensor.matmul(
                                    pbv(jj),
                                    wg[:, dmc, (j0 + jj) * P:(j0 + jj + 1) * P],
                                    xt[:, dmc, :],
                                    start=(dmc == 0 and jj % 2 == 0),
                                    stop=(dmc == NDM - 1),
                                    skip_group_check=True)
                        for jj in range(0, 4, 2):
                            jc = grp * 8 + j0 + jj
                            # u = SHSY*B' (pairs); bps copies u; ga8 per chunk
                            nc.vector.tensor_scalar(
                                u[:, jc:jc + 2, :], pbs[jj // 2][:], SHSY,
                                None, op0=Alu.mult)
                            nc.gpsimd.tensor_copy(bps[:, jc:jc + 2, :],
                                                  u[:, jc:jc + 2, :])
                            for q in range(2):
                                nc.scalar.activation(
                                    ga8[:, jc + q, :], pbv(jj + q), Act.Relu,
                                    bias=gbi[:, jc + q:jc + q + 1],
                                    scale=gsb[:, jc + q:jc + q + 1])

                # ---- standalone ht for step 1 ----
                if nsteps:
                    ht8 = htp.tile([P, NDM, T], dt.float8e4, tag="ht8")
                phs = []
                for _ in range(4 if nsteps else 0):
                    ph = apsum.tile([P, 2 * T], dt.float32, tag="mm")
                    phs.append(ph)
                def phv(dmc):
                    return phs[dmc // 2][:, (dmc % 2) * T:(dmc % 2 + 1) * T]
                for p in range(NPAIR_H if nsteps else 0):
                    for dmc in range(NDM):
                        nc.tensor.matmul(
                            phv(dmc), wti[:, p, dmc, :],
                            ga8[:, 2 * p:2 * p + 2, :],
                            start=(p == 0 and dmc % 2 == 0),
                            stop=(p == NPAIR_H - 1),
                            perf_mode=SWI, skip_group_check=True)
                for dmc in range(NDM if nsteps else 0):
                    if dmc % 2 == 0:
                        nc.scalar.activation(ht8[:, dmc, :], phv(dmc),
                                             Act.Copy, scale=C_HT)
                    else:
                        nc.vector.tensor_scalar(ht8[:, dmc, :], phv(dmc),
                                                C_HT, None, op0=Alu.mult)

                # ---- iterated steps; ht(k+1) fused into Y(k) pair loop ----
                for k in range(1, nsteps + 1):
                    last = (k == nsteps)
                    if not last:
                        ht8_n = htp.tile([P, NDM, T], dt.float8e4, tag="ht8")
                        phs_n = []
                        for _ in range(4):
                            ph = apsum.tile([P, 2 * T], dt.float32, tag="mm")
                            phs_n.append(ph)

                    LAGP = 2
                    ss = {}

                    def emit_upd(j):
                        jc = 2 * j
                        # u' = 0.9*u + s  on the pair (DVE STT, lagged)
                        nc.vector.scalar_tensor_tensor(
                            u[:, jc:jc + 2, :], u[:, jc:jc + 2, :], 0.9,
                            ss[j][:], op0=Alu.mult, op1=Alu.add)
                        if not last:
                            for q in range(2):
                                nc.scalar.activation(
                                    ga8[:, jc + q, :], u[:, jc + q, :],
                                    Act.Relu,
                                    bias=gbi[:, jc + q:jc + q + 1],
                                    scale=gss[:, jc + q:jc + q + 1])
                            final = (j == NPAIR_H - 1)
                            for dmc in range(NDM):
                                nc.tensor.matmul(
                                    phs_n[dmc // 2][:, (dmc % 2) * T:
                                                    (dmc % 2 + 1) * T],
                                    wti[:, j, dmc, :],
                                    ga8[:, jc:jc + 2, :],
                                    start=(j == 0 and dmc % 2 == 0),
                                    stop=final,
                                    perf_mode=SWI, skip_group_check=True)
                                if final and dmc % 2 == 1:
                                    pv0 = phs_n[(dmc - 1) // 2][
                                        :, ((dmc - 1) % 2) * T:
                                        ((dmc - 1) % 2 + 1) * T]
                                    pv1 = phs_n[dmc // 2][
                                        :, (dmc % 2) * T:(dmc % 2 + 1) * T]
                                    nc.scalar.activation(
                                        ht8_n[:, dmc - 1, :], pv0,
                                        Act.Copy, scale=C_HT)
                                    nc.vector.tensor_scalar(
                                        ht8_n[:, dmc, :], pv1, C_HT, None,
                                        op0=Alu.mult)

                    for j in range(NPAIR_H):
                        py2 = ypsum.tile([P, 2 * T], dt.float32, tag="ymm")
                        for q in range(2):
                            jc = 2 * j + q
                            py = py2[:, q * T:(q + 1) * T]
                            for d in range(NPAIR_Y):
                                nc.tensor.matmul(
                                    py, w01i[:, d, jc, :],
                                    ht8[:, 2 * d:2 * d + 2, :],
                                    start=(d == 0 and q == 0),
                                    stop=(d == NPAIR_Y - 1),
                                    perf_mode=SWI, skip_group_check=True)
                        # t = (ga8 * C_T) + py2 = SHSY*(g*a - Y') on the pair
                        t = ttmp.tile([P, 2 * T], dt.bfloat16, tag="t")
                        nc.vector.scalar_tensor_tensor(
                            t[:], ga8[:, 2 * j:2 * j + 2, :], C_T, py2[:],
                            op0=Alu.mult, op1=Alu.add)
                        # s = t + bps   (Pool)
                        s = stmp.tile([P, 2 * T], dt.bfloat16, tag="s")
                        nc.gpsimd.tensor_tensor(
                            s[:], t[:], bps[:, 2 * j:2 * j + 2, :], op=Alu.add)
                        ss[j] = s
                        if j >= LAGP:
                            emit_upd(j - LAGP)
                    for j in range(NPAIR_H - LAGP, NPAIR_H):
                        emit_upd(j)

                    if not last:
                        ht8 = ht8_n

                # ---- out = relu(u - 0.1) @ W^T in bf16, weights streamed ----
                ab = state.tile([P, NDL, T], dt.bfloat16, tag="ab")
                pos = []
                for _ in range(4):
                    po = ypsum.tile([P, 512], dt.float32, tag="ymm")
                    pos.append(po)
                for c in range(8):   # chunks of 4 kc
                    ws = wtbs.tile([P, 4, DM], dt.bfloat16, tag="ws")
                    for h in range(4):
                        nc.sync.dma_start(ws[:, h], wtb_d[:, 4 * c + h])
                    for h in range(4):
                        kc = 4 * c + h
                        nc.scalar.activation(ab[:, kc, :], u[:, kc, :],
                                             Act.Relu, bias=nl_ab[:, 0:1],
                                             scale=1.0 / SHSY)
                        for sub in range(2):
                            for nh in range(2):
                                nc.tensor.matmul(
                                    pos[sub * 2 + nh][:],
                                    ab[:, kc, sub * P:(sub + 1) * P],
                                    ws[:, h, nh * 512:(nh + 1) * 512],
                                    start=(kc == 0), stop=(kc == NDL - 1))
                for sub in range(2):
                    ob = oio.tile([P, DM], dt.float32, tag="ob")
                    nc.vector.tensor_copy(ob[:, 0:512], pos[sub * 2][:])
                    nc.scalar.copy(ob[:, 512:1024], pos[sub * 2 + 1][:])
                    row = blk * T + sub * P
                    nc.sync.dma_start(out_d[row:row + P, :], ob[:])

    nc.compile()
    return nc


def _get_nc(nsteps=NSTEPS, nrep=1):
    key = ("nc", nsteps, nrep)
    if key not in _CACHE:
        _CACHE[key] = _build_nc(nsteps, nrep)
    return _CACHE[key]


def _swi_interleave(lhsT0, lhsT1):
    """lhsT_i: [..., K, M] logical stationary halves -> SWI layout [..., K, 2M]
    with w[..., k, 2j] = lhsT0[..., k, M-1-j], w[..., k, 2j+1] = lhsT1[..., k, M-1-j]."""
    M = lhsT0.shape[-1]
    out = np.empty(lhsT0.shape[:-1] + (2 * M,), np.float32)
    out[..., 0::2] = lhsT0[..., ::-1]
    out[..., 1::2] = lhsT1[..., ::-1]
    return out


def _prep_shared(W):
    W = np.asarray(W, np.float32)
    g = 0.1 * (W.astype(np.float64) ** 2).sum(0)    # [DL]
    gf = g.astype(np.float32)

    # wti: SWI stationary for ht, with 1/g folded per dl row.
    # Logical half i of pair p, dm chunk c: lhsT[k, m] =
    #   SW2 * W[c*128+m, (2p+i)*128+k] / g[(2p+i)*128+k]
    Wg = SW2 * (W / gf[None, :])
    A = Wg.reshape(NDM, P, NPAIR_H, 2, P)             # [c, m, p, i, k]
    A = A.transpose(2, 3, 0, 4, 1)                    # [p, i, c, k, m]
    wti = _swi_interleave(A[:, 0], A[:, 1])           # [p, c, k, 2P]
    wti = np.ascontiguousarray(wti.transpose(2, 0, 1, 3)).astype(F8)

    # w01i: SWI stationary for Y, negated: lhsT[k, m] =
    #   -SY*0.1*W[(2d+i)*128+k, jc*128+m]
    B = (-SY * 0.1 * W).reshape(NPAIR_Y, 2, P, NDL, P)  # [d, i, k, jc, m]
    B = B.transpose(0, 1, 3, 2, 4)                      # [d, i, jc, k, m]
    w01i = _swi_interleave(B[:, 0], B[:, 1])            # [d, jc, k, 2P]
    w01i = np.ascontiguousarray(w01i.transpose(2, 0, 1, 3)).astype(F8)

    # w01b: bf16 B'-phase stationary stream: [k, grp, dmc, c1024]
    C = (0.1 * W).reshape(NDM, P, 4, 1024)            # [dmc, k, grp, c]
    w01b = np.ascontiguousarray(C.transpose(1, 2, 0, 3)).astype(BF16)

    # wtb: bf16 out-phase moving stream: [k, kc, dm] = W[dm, kc*128+k]
    D = W.T.reshape(NDL, P, DM)                       # [kc, k, dm]
    wtb = np.ascontiguousarray(D.transpose(1, 0, 2)).astype(BF16)

    # per-(partition, chunk) ACT scale/bias vectors
    gmat = gf.reshape(NDL, P).T                        # [P, NDL]
    gsb = np.ascontiguousarray(SGA * gmat).astype(np.float32)
    gss = np.ascontiguousarray(SGA * gmat / SHSY).astype(np.float32)
    gbi = np.ascontiguousarray(-0.1 * SGA * gmat).astype(np.float32)
    return wti, w01i, w01b, wtb, gsb, gss, gbi


def kernel(x, W):
    import os

    from concourse.bass_utils import run_bass_kernel_spmd

    if not TRACE:
        os.environ.setdefault("BASS_NEVER_TRACE", "1")
    x = np.asarray(x)
    orig_shape = x.shape
    xf = x.reshape(-1, DM).astype(np.float32)
    wti, w01i, w01b, wtb, gsb, gss, gbi = _prep_shared(W)

    in_maps = []
    for c in range(NCORES):
        xs = xf[c * TOK_CORE:(c + 1) * TOK_CORE]
        xt = np.ascontiguousarray(
            xs.reshape(NBLK, T, NDM, P).transpose(0, 3, 2, 1)).astype(BF16)
        in_maps.append({"xt": xt, "wti": wti, "w01i": w01i,
                        "w01b": w01b, "wtb": wtb,
                        "gsb": gsb, "gss": gss, "gbi": gbi})

    nc = _get_nc()
    res = run_bass_kernel_spmd(nc, in_maps, core_ids=list(range(NCORES)),
                               trace=TRACE)
    global LAST_RESULT
    LAST_RESULT = res
    out = np.concatenate([res.results[c]["out"] for c in range(NCORES)], axis=0)
    return out.reshape(orig_shape).astype(np.float32)

